# revision 1
# baseline (speedup 1.0000x reference)
"""Trainium2 Bass kernel for nn_CrossAttention (B=4, C=512, H=W=64, CQK=64).

Math (per batch b):
    Q = Wq @ rgb + bq                      [CQK, HW]
    K = Wk @ chm + bk                      [CQK, XY]
    V = Wv @ chm + bv                      [C, XY]
    S[hw, xy] = sum_o Q[o, hw] K[o, xy]    (xy = x*64 + y)
    P = softmax over y only (last 64-group of xy)
    att[c, hw] = sum_xy P[hw, xy] V[c, xy]
    out = rgb + gamma * att

Sharding: 8 cores = 4 batches x 2 halves of the hw (query) axis. Weights
replicated. Each core computes the full K/V for its batch and its 2048-row
slice of queries.

Device dataflow per core (all layouts channel/feature-major):
  - Qt[o, hw] (f32r), Kf[o, xy] (f32r) via 1x1-conv GEMMs; V^T[xy, c] (bf16).
  - S tiles [128 hw, xy] on PE (f32r), exp on ACT -> P (bf16, unnormalized),
    Z via DVE pairwise-tree sum over y, reciprocal, broadcast-multiply.
  - P^T via DMA xbar transpose (bf16), attend GEMM on PE (bf16),
    final add with rgb on DVE.
  - gamma and bv are folded on the host (bv contributes 64*gamma*bv[c] since
    softmax rows sum to 1 per (hw, x) and there are 64 x's).
DMA engine split: bulk loads on SWDGE (gpsimd), stores on the ACT HWDGE ring
(scalar), xbar transposes on the SP HWDGE ring (sync) to avoid single-FIFO
serialization.
"""

import numpy as np

import concourse.bass as bass
import concourse.mybir as mybir
import concourse.tile as tile
from concourse import bacc
from concourse.bass_utils import run_bass_kernel_spmd

P = 128
B, C, H, W = 4, 512, 64, 64
HW = H * W                # 4096
CQK = C // 8              # 64
N_CORES = 8
HWC = HW // 2             # hw rows per core (2048)

F32 = mybir.dt.float32
F32R = mybir.dt.float32r
BF16 = mybir.dt.bfloat16
ADD = mybir.AluOpType.add
MULT = mybir.AluOpType.mult
IDENT = mybir.ActivationFunctionType.Identity
EXP = mybir.ActivationFunctionType.Exp


def build_program(hwc=HWC, xy=HW, c=C, cqk=CQK, n_cores=N_CORES, repeat=1,
                  load_eng="gpsimd", store_eng="sync", ptb_bufs=2):
    """Build the per-core Bass program. Returns a compiled Bacc module."""
    ck = c // P               # channel chunks (4)
    nb = hwc // 512           # hw blocks (4)
    xt = xy // P              # xy tiles (32)
    xb = xy // 512            # xy 512-blocks (8)
    y = 64                    # softmax group size
    x_per_tile = xy // y      # x values (64 full size)

    nc = bacc.Bacc("TRN2", target_bir_lowering=False, debug=False,
                   num_devices=n_cores)
    ld = {"sync": nc.sync, "scalar": nc.scalar, "gpsimd": nc.gpsimd}[load_eng]
    st = {"sync": nc.sync, "scalar": nc.scalar, "gpsimd": nc.gpsimd}[store_eng]

    rgb = nc.dram_tensor("rgb", [c, hwc], F32, kind="ExternalInput")
    chm = nc.dram_tensor("chm", [c, xy], F32, kind="ExternalInput")
    wqT = nc.dram_tensor("wqT", [c, 2 * cqk], F32, kind="ExternalInput")
    wkT = nc.dram_tensor("wkT", [c, 2 * cqk], F32, kind="ExternalInput")
    wvT = nc.dram_tensor("wvT", [c, c], F32, kind="ExternalInput")
    bq = nc.dram_tensor("bq", [2 * cqk, 1], F32, kind="ExternalInput")
    bk = nc.dram_tensor("bk", [2 * cqk, 1], F32, kind="ExternalInput")
    out = nc.dram_tensor("out", [c, hwc], F32, kind="ExternalOutput")

    rgb_t = rgb.ap().rearrange("(k p) n -> p k n", p=P)
    chm_t = chm.ap().rearrange("(k p) n -> p k n", p=P)
    wq_t = wqT.ap().rearrange("(k p) m -> p k m", p=P)
    wk_t = wkT.ap().rearrange("(k p) m -> p k m", p=P)
    wv_t = wvT.ap().rearrange("(k p) m -> p k m", p=P)
    out_t = out.ap().rearrange("(k p) n -> p k n", p=P)

    with tile.TileContext(nc) as tc:
        with tc.tile_pool(name="persist", bufs=1) as pers:
            # --- weights / biases ---
            wq_r = pers.tile([P, ck, 2 * cqk], F32R)
            wk_r = pers.tile([P, ck, 2 * cqk], F32R)
            wv_b = pers.tile([P, ck, c], BF16)
            with tc.tile_pool(name="wload", bufs=1) as wload:
                wq_f = wload.tile([P, ck, 2 * cqk], F32)
                ld.dma_start(wq_f[:], wq_t)
                nc.vector.tensor_copy(wq_r[:], wq_f[:])
            bq_sb = pers.tile([2 * cqk, 1], F32)
            ld.dma_start(bq_sb[:], bq.ap())
            bk_sb = pers.tile([2 * cqk, 1], F32)
            ld.dma_start(bk_sb[:], bk.ap())

            qt_sb = pers.tile([2 * cqk, hwc], F32R)
            kf_sb = pers.tile([2 * cqk, xy], F32R)
            chmT_bf = pers.tile([P, xt, ck, P], BF16)

            for _rep in range(repeat):
                # deferred weight loads (not needed until Kf / att2)
                with tc.tile_pool(name="wload2", bufs=1) as wload2:
                    wk_f = wload2.tile([P, ck, 2 * cqk], F32)
                    ld.dma_start(wk_f[:], wk_t)
                    nc.vector.tensor_copy(wk_r[:], wk_f[:])
                    wv_f = wload2.tile([P, ck, c], F32)
                    ld.dma_start(wv_f[:], wv_t)
                    nc.vector.tensor_copy(wv_b[:], wv_f[:])

                # --- phase 1: Qt GEMM (rgb streamed) then Kf GEMM (chm
                # streamed); chm_bf shares the ptb tag: identical
                # 32KB/partition footprint, so phase 2's P^T buffers reuse its
                # slot once the chmT transposes are done.
                with tc.tile_pool(name="ptpool", bufs=ptb_bufs) as ptpool:
                    chm_bf = ptpool.tile([P, ck, xy], BF16, tag="ptb",
                                         name="chmbf")
                    half = xy // 2
                    with tc.tile_pool(name="qstream", bufs=2) as qstream, \
                         tc.tile_pool(name="psQ", bufs=1, space="PSUM") as psQ:
                        q_ps = [psQ.tile([2 * cqk, 512], F32, name=f"qps{i}")
                                for i in range(nb)]
                        for k in range(ck):
                            rf = qstream.tile([P, hwc], F32, tag="rf")
                            ld.dma_start(rf[:], rgb_t[:, k])
                            rr = qstream.tile([P, hwc], F32R, tag="rr")
                            nc.vector.tensor_copy(rr[:], rf[:])
                            for j in range(nb):
                                nc.tensor.matmul(
                                    q_ps[j][:], wq_r[:, k],
                                    rr[:, 512 * j:512 * (j + 1)],
                                    start=(k == 0), stop=(k == ck - 1))
                        for i in range(nb):
                            nc.scalar.activation(qt_sb[:, 512 * i:512 * (i + 1)],
                                                 q_ps[i][:], IDENT, bias=bq_sb[:])

                    with tc.tile_pool(name="stream", bufs=2) as stream, \
                         tc.tile_pool(name="psK", bufs=1, space="PSUM") as psK:
                        k_ps = [psK.tile([2 * cqk, 512], F32, name=f"kps{i}")
                                for i in range(xb)]
                        for k in range(ck):
                            for h in range(2):
                                cf = stream.tile([P, half], F32, tag="cf")
                                ld.dma_start(
                                    cf[:], chm_t[:, k, h * half:(h + 1) * half])
                                nc.scalar.copy(
                                    chm_bf[:, k, h * half:(h + 1) * half], cf[:])
                                cr = stream.tile([P, half], F32R, tag="cr")
                                nc.vector.tensor_copy(cr[:], cf[:])
                                for j in range(xb // 2):
                                    xblk = h * (xb // 2) + j
                                    nc.tensor.matmul(
                                        k_ps[xblk][:], wk_r[:, k],
                                        cr[:, 512 * j:512 * (j + 1)],
                                        start=(k == 0), stop=(k == ck - 1))
                        for i in range(xb):
                            nc.scalar.activation(kf_sb[:, 512 * i:512 * (i + 1)],
                                                 k_ps[i][:], IDENT, bias=bk_sb[:])

                    # chmT transposes: deferred so they fill DMA idle slots
                    # during the first softmax block (M1 needs them later).
                    for k in range(ck):
                        nc.sync.dma_start(chmT_bf[:, :, k, :], chm_bf[:, k],
                                          transpose=True)

                    # --- phase 2 (software-pipelined with V^T):
                    #     softmax(0) | V^T | softmax(b+1) interleaved with
                    #     attend(b) so PE fills gaps while ACT/DVE work ahead.
                    with tc.tile_pool(name="pmain", bufs=3) as pmain, \
                         tc.tile_pool(name="zpool", bufs=1) as zpool, \
                         tc.tile_pool(name="rgbf", bufs=1) as rgbf, \
                         tc.tile_pool(name="opool", bufs=2) as opool, \
                         tc.tile_pool(name="m1pool", bufs=2) as m1pool, \
                         tc.tile_pool(name="psS", bufs=2, space="PSUM") as psS, \
                         tc.tile_pool(name="psA", bufs=2, space="PSUM") as psA, \
                         nc.allow_low_precision(reason="softmax weights in bf16"):

                        def softmax_block(blk):
                            ptb = ptpool.tile([P, 4, xt, P], BF16, tag="ptb",
                                              name=f"ptb{blk}")
                            for ht in range(4):
                                htile = blk * 4 + ht
                                p_sb = pmain.tile([P, xy], BF16, tag="p")
                                for s in range(xy // 1024):
                                    s_ps = psS.tile([P, 1024], F32, tag="sps")
                                    # two K=64 matmuls packed into disjoint PE
                                    # row groups run concurrently in the array
                                    nc.tensor.matmul(
                                        s_ps[:, 0:512],
                                        qt_sb[0:cqk, P * htile:P * (htile + 1)],
                                        kf_sb[0:cqk, 1024 * s:1024 * s + 512],
                                        start=True, stop=True,
                                        tile_position=(0, 0))
                                    nc.tensor.matmul(
                                        s_ps[:, 512:1024],
                                        qt_sb[cqk:2 * cqk, P * htile:P * (htile + 1)],
                                        kf_sb[cqk:2 * cqk, 1024 * s + 512:1024 * (s + 1)],
                                        start=True, stop=True,
                                        tile_position=(cqk, 0))
                                    nc.scalar.activation(
                                        p_sb[:, 1024 * s:1024 * (s + 1)], s_ps[:], EXP)
                                # Z = sum over y (pairwise tree, bf16)
                                v3 = p_sb[:].rearrange("p (x y) -> p x y", y=y)
                                tcur = v3
                                w = y
                                while w > 1:
                                    w //= 2
                                    tnext = zpool.tile([P, x_per_tile, w], BF16,
                                                       tag=f"z{w}")
                                    nc.vector.tensor_tensor(
                                        tnext[:], tcur[:, :, 0:w], tcur[:, :, w:2 * w],
                                        ADD)
                                    tcur = tnext
                                rz = zpool.tile([P, x_per_tile, 1], BF16, tag="rz")
                                nc.vector.reciprocal(rz[:], tcur[:])
                                nc.vector.tensor_tensor(
                                    v3, v3, rz[:].to_broadcast([P, x_per_tile, y]),
                                    MULT)
                                nc.sync.dma_start(ptb[:, ht], p_sb[:], transpose=True)
                            return ptb

                        def attend_block(blk, ptb):
                            rg = rgbf.tile([P, ck, 512], F32, tag="rg")
                            ld.dma_start(rg[:],
                                         rgb_t[:, :, 512 * blk:512 * (blk + 1)])
                            # M1[cin, hw] = sum_xy chm[cin, xy] P^T[xy, hw]
                            m1_sb = m1pool.tile([P, ck, 512], BF16, tag="m1")
                            for ch in range(ck):
                                m_ps = psA.tile([P, 512], F32, tag="aps")
                                for m in range(xt):
                                    nc.tensor.matmul(
                                        m_ps[:], chmT_bf[:, m, ch, :],
                                        ptb[:, :, m, :],
                                        start=(m == 0), stop=(m == xt - 1))
                                nc.vector.tensor_copy(m1_sb[:, ch], m_ps[:])
                            # att[c, hw] = sum_cin (gamma Wv)[c, cin] M1[cin, hw]
                            o_sb = opool.tile([P, ck, 512], F32, tag="o")
                            for ct in range(ck):
                                a_ps = psA.tile([P, 512], F32, tag="aps")
                                for ch in range(ck):
                                    nc.tensor.matmul(
                                        a_ps[:], wv_b[:, ch, P * ct:P * (ct + 1)],
                                        m1_sb[:, ch],
                                        start=(ch == 0), stop=(ch == ck - 1))
                                nc.vector.tensor_tensor(o_sb[:, ct], a_ps[:],
                                                        rg[:, ct], ADD)
                            st.dma_start(out_t[:, :, 512 * blk:512 * (blk + 1)],
                                         o_sb[:])

                        ptbs = {0: softmax_block(0)}

                        for blk in range(1, nb):
                            ptbs[blk] = softmax_block(blk)
                            attend_block(blk - 1, ptbs.pop(blk - 1))
                        attend_block(nb - 1, ptbs.pop(nb - 1))

    nc.compile()
    return nc


_NC_CACHE = {}


def _get_nc():
    if "nc" not in _NC_CACHE:
        _NC_CACHE["nc"] = build_program()
    return _NC_CACHE["nc"]


def make_in_maps(rgb_features, chm_features, Wq, bq, Wk, bk, Wv, bv, gamma):
    rgb_features = np.asarray(rgb_features, dtype=np.float32)
    chm_features = np.asarray(chm_features, dtype=np.float32)
    Wq = np.asarray(Wq, dtype=np.float32)
    Wk = np.asarray(Wk, dtype=np.float32)
    Wv = np.asarray(Wv, dtype=np.float32)
    bq = np.asarray(bq, dtype=np.float32)
    bk = np.asarray(bk, dtype=np.float32)
    bv = np.asarray(bv, dtype=np.float32)
    g = float(np.asarray(gamma).reshape(-1)[0])

    wqT = np.ascontiguousarray(np.concatenate([Wq.T, Wq.T], axis=1))
    wkT = np.ascontiguousarray(np.concatenate([Wk.T, Wk.T], axis=1))
    wvT = np.ascontiguousarray((g * Wv).T)
    # softmax rows sum to 1 per (hw, x); summing over the 64 x's makes the
    # bias term contribute exactly 64*gamma*bv[c] to every output pixel.
    rgb_adj = rgb_features + (64.0 * g * bv)[None, :, None, None]
    bq2 = np.ascontiguousarray(np.concatenate([bq, bq]).reshape(2 * CQK, 1))
    bk2 = np.ascontiguousarray(np.concatenate([bk, bk]).reshape(2 * CQK, 1))

    in_maps = []
    for core in range(N_CORES):
        b, half = divmod(core, 2)
        rgb_c = np.ascontiguousarray(
            rgb_adj[b].reshape(C, HW)[:, half * HWC:(half + 1) * HWC])
        chm_c = np.ascontiguousarray(chm_features[b].reshape(C, HW))
        in_maps.append({
            "rgb": rgb_c, "chm": chm_c,
            "wqT": wqT, "wkT": wkT, "wvT": wvT,
            "bq": bq2, "bk": bk2,
        })
    return in_maps


def assemble(results):
    fused = np.empty((B, C, H, W), dtype=np.float32)
    fused2 = fused.reshape(B, C, HW)
    for core in range(N_CORES):
        b, half = divmod(core, 2)
        fused2[b, :, half * HWC:(half + 1) * HWC] = results[core]["out"]
    return fused


def kernel(rgb_features, chm_features, Wq, bq, Wk, bk, Wv, bv, gamma):
    nc = _get_nc()
    in_maps = make_in_maps(rgb_features, chm_features, Wq, bq, Wk, bk, Wv, bv,
                           gamma)
    res = run_bass_kernel_spmd(nc, in_maps, core_ids=list(range(N_CORES)))
    return assemble(res.results)

